# revision 1
# baseline (speedup 1.0000x reference)
"""Trainium2 Bass kernel for the Capsule routing module.

Computation (per batch b):
    u_hat = u_vecs[b] @ W[0]            # (512, 2048), o = n*64 + d
    3 rounds of dynamic routing over 32 capsules of dim 64:
        c = softmax_n(b_logits); v = squash(sum_i c * u_hat); b_logits = v . u_hat
Returns outputs (64, 32, 64).

Sharding: data-parallel over batch across 8 NeuronCores (8 batches/core),
W replicated.  All matmuls in bf16 with fp32 PSUM accumulation.

Host-side prep (zero FLOPs): per-batch transpose of u_vecs (so the
contraction dim k lands on SBUF partitions), W transpose copy, and a small
constants tensor (identity / uniform-c / block-diag masks).
"""

import numpy as np

NUM_CAP = 32
DIM_CAP = 64
ROUTINGS = 3
EPS = 1e-7
B, S, D_IN = 64, 512, 512
D_OUT = NUM_CAP * DIM_CAP  # 2048
N_CORES = 8
BPC = B // N_CORES  # batches per core

_PROGRAM_CACHE = {}
PIPELINE = True
EXP_SCALE_AP = True


def _build_consts_np():
    """[128, 416] bf16 constants: I128 | cT0 (1/32) | MASKBIG."""
    import ml_dtypes

    c = np.zeros((128, 416), np.float32)
    c[:, :128] = np.eye(128, dtype=np.float32)
    c[:, 128:160] = 1.0 / NUM_CAP
    # MASKBIG[p, 16*oc + m] = 1 iff m == 2*(oc % 8) + p // 64
    for oc in range(16):
        for j in (0, 1):
            m = 2 * (oc % 8) + j
            c[64 * j : 64 * (j + 1), 160 + 16 * oc + m] = 1.0
    return c.astype(ml_dtypes.bfloat16)


def _build_program():
    import concourse.bass as bass
    import concourse.mybir as mybir
    from concourse import bacc
    from concourse.tile import TileContext

    f32 = mybir.dt.float32
    bf16 = mybir.dt.bfloat16
    AF = mybir.ActivationFunctionType
    ALU = mybir.AluOpType

    nc = bacc.Bacc("TRN2", debug=False)

    uT_d = nc.dram_tensor("uT", [BPC, D_IN, S], f32, kind="ExternalInput")
    W_d = nc.dram_tensor("W", [D_IN, D_OUT], f32, kind="ExternalInput")
    WT_d = nc.dram_tensor("WT", [D_OUT, D_IN], f32, kind="ExternalInput")
    consts_d = nc.dram_tensor("consts", [128, 416], bf16, kind="ExternalInput")
    out_d = nc.dram_tensor("out", [BPC, NUM_CAP, DIM_CAP], f32, kind="ExternalOutput")

    KC = D_IN // 128   # 4 k-chunks
    IC = S // 128      # 4 i-chunks
    OC = D_OUT // 512  # 4 o-chunks of 512
    OT = D_OUT // 128  # 16 o-partition-tiles

    with TileContext(nc) as tc:
        with (
            tc.tile_pool(name="persist", bufs=1) as ppool,
            tc.tile_pool(name="ut", bufs=4) as utpool,
            tc.tile_pool(name="uh", bufs=4) as uhpool,
            tc.tile_pool(name="small", bufs=4) as spool,
            tc.tile_pool(name="pp", bufs=4, space="PSUM") as pp_pool,
            tc.tile_pool(name="ps", bufs=1, space="PSUM") as ps_pool,
            tc.tile_pool(name="phb", bufs=1, space="PSUM") as phb_pool,
            tc.tile_pool(name="pt", bufs=2, space="PSUM") as pt_pool,
            tc.tile_pool(name="dram", bufs=2, space="DRAM") as dram_pool,
        ):
            # ---- once-per-core loads -------------------------------------
            consts = ppool.tile([128, 416], bf16, tag="consts", name="consts")
            nc.sync.dma_start(out=consts[:], in_=consts_d[:])
            I128 = consts[:, 0:128]
            CT0 = consts[:, 128:160]
            MASKBIG = consts[:, 160:416]

            W_bf = []
            for kc in range(KC):
                t = ppool.tile([128, D_OUT], bf16, tag=f"wbf{kc}", name=f"wbf{kc}")
                nc.gpsimd.dma_start(out=t[:], in_=W_d[128 * kc : 128 * (kc + 1), :])
                W_bf.append(t)

            WT_bf = []
            for ot in range(OT):
                t = ppool.tile([128, D_IN], bf16, tag=f"wt{ot}", name=f"wt{ot}")
                nc.gpsimd.dma_start(out=t[:], in_=WT_d[128 * ot : 128 * (ot + 1), :])
                WT_bf.append(t)

            def emit_loads(b):
                uT = []
                for kc in range(KC):
                    t = utpool.tile([128, S], bf16, tag=f"ut{kc}", name=f"ut{kc}")
                    nc.gpsimd.dma_start(
                        out=t[:], in_=uT_d[b, 128 * kc : 128 * (kc + 1), :]
                    )
                    uT.append(t)
                uh = [
                    uhpool.tile([128, D_OUT], bf16, tag=f"uh{ic}", name=f"uh{ic}")
                    for ic in range(IC)
                ]
                return uT, uh

            def proj_chunks(uT, u_hat):
                """4 closures (one per ic): kc-outer loop reuses each
                stationary for 4 consecutive matmuls into 4 psum banks."""

                def mk(ic):
                    def go():
                        pps = [
                            pp_pool.tile([128, 512], f32, tag="pp", name="pp")
                            for _ in range(OC)
                        ]
                        for kc in range(KC):
                            for oc in range(OC):
                                nc.tensor.matmul(
                                    pps[oc][:],
                                    uT[kc][:, 128 * ic : 128 * (ic + 1)],
                                    W_bf[kc][:, 512 * oc : 512 * (oc + 1)],
                                    start=(kc == 0),
                                    stop=(kc == KC - 1),
                                )
                        for oc in range(OC):
                            nc.vector.tensor_copy(
                                u_hat[ic][:, 512 * oc : 512 * (oc + 1)], pps[oc][:]
                            )

                    return go

                return [mk(ic) for ic in range(IC)]

            def routing(b, uT, u_hat):
                """Generator: yields at pipeline stall points (phase ends)."""
                cT = None
                for it in range(ROUTINGS):
                    # --- PH_A: per-capsule weighted sums, 4x col-packed ----
                    # n = 8q + r lands at psum row 32q, cols 64r:64r+64; only
                    # the diagonal block is ever computed (M=1 matmuls, four
                    # concurrent via distinct col groups).
                    S_sb = spool.tile(
                        [NUM_CAP, DIM_CAP], f32, tag="S_sb", name="S_sb"
                    )
                    psS = ps_pool.tile([128, 512], f32, tag="ps", name="psS")
                    for r in range(8):
                        for ic in range(IC):
                            lhsT = CT0 if cT is None else cT[ic]
                            for q in range(4):
                                n = 8 * q + r
                                nc.tensor.matmul(
                                    psS[32 * q : 32 * q + 1, 64 * r : 64 * (r + 1)],
                                    lhsT[:, n : n + 1],
                                    u_hat[ic][:, 64 * n : 64 * (n + 1)],
                                    start=(ic == 0),
                                    stop=(ic == IC - 1),
                                    tile_position=(0, 32 * q),
                                )
                    s_sc = spool.tile([128, 512], f32, tag="s_sc", name="s_sc")
                    for q in range(4):
                        nc.scalar.copy(
                            out=s_sc[32 * q : 32 * q + 1, :],
                            in_=psS[32 * q : 32 * q + 1, :],
                        )
                    # single SBUF->SBUF DMA de-scatters to [32, 64]
                    sc_src = bass.AP(
                        s_sc.tensor,
                        s_sc.offset,
                        [[32 * s_sc.ap[0][0], 4], [DIM_CAP, 8], [1, DIM_CAP]],
                    )
                    nc.sync.dma_start(out=S_sb[:], in_=sc_src)
                    yield

                    # --- PH_B: squash stats --------------------------------
                    sq_junk = spool.tile(
                        [NUM_CAP, DIM_CAP], f32, tag="sq_junk", name="sq_junk"
                    )
                    ss = spool.tile([NUM_CAP, 1], f32, tag="ss", name="ss")
                    nc.vector.tensor_tensor(sq_junk[:], S_sb[:], S_sb[:], op=ALU.mult)
                    nc.vector.reduce_sum(ss[:], sq_junk[:], axis=mybir.AxisListType.X)
                    ss_eps = spool.tile([NUM_CAP, 1], f32, tag="ss_eps", name="ss_eps")
                    nc.vector.tensor_scalar_add(ss_eps[:], ss[:], EPS)
                    rinv = spool.tile([NUM_CAP, 1], f32, tag="rinv", name="rinv")
                    nc.vector.reciprocal(rinv[:], ss_eps[:])
                    rs = spool.tile([NUM_CAP, 1], f32, tag="rs", name="rs")
                    nc.scalar.activation(out=rs[:], in_=rinv[:], func=AF.Sqrt)

                    if it == ROUTINGS - 1:
                        v_f32 = spool.tile(
                            [NUM_CAP, DIM_CAP], f32, tag="v_f32", name="v_f32"
                        )
                        nc.vector.tensor_scalar_mul(v_f32[:], S_sb[:], rs[:])
                        nc.sync.dma_start(out=out_d[b], in_=v_f32[:])
                        return

                    # unnormalized S in bf16; rs folds into the exp scale
                    S_bf = spool.tile(
                        [NUM_CAP, DIM_CAP], bf16, tag="S_bf", name="S_bf"
                    )
                    nc.vector.tensor_copy(S_bf[:], S_sb[:])
                    yield

                    # --- PH_C: v_stat / v_diag -----------------------------
                    pvt = pt_pool.tile([DIM_CAP, NUM_CAP], bf16, tag="pt", name="pvt")
                    nc.tensor.transpose(pvt[:], S_bf[:], I128[0:NUM_CAP, 0:NUM_CAP])
                    vT_sb = spool.tile(
                        [DIM_CAP, NUM_CAP], bf16, tag="vT_sb", name="vT_sb"
                    )
                    nc.scalar.copy(out=vT_sb[:], in_=pvt[:])
                    v_stat = spool.tile([128, 16], bf16, tag="v_stat", name="v_stat")
                    vT_pair = vT_sb.rearrange("p (o two) -> p two o", two=2)
                    nc.vector.tensor_copy(v_stat[0:64, :], vT_pair[:, 0, :])
                    # partition-shifting copy (rows 0:64 -> 64:128): DMA only
                    nc.sync.dma_start(out=v_stat[64:128, :], in_=vT_pair[:, 1, :])

                    v_diag = spool.tile([128, 256], bf16, tag="v_diag", name="v_diag")
                    mask3 = bass.AP(
                        MASKBIG.tensor,
                        MASKBIG.offset,
                        [MASKBIG.ap[0], [16, 16], [1, 16]],
                    )
                    vstat3 = bass.AP(
                        v_stat.tensor,
                        v_stat.offset,
                        [v_stat.ap[0], [1, 16], [0, 16]],
                    )
                    vdiag3 = bass.AP(
                        v_diag.tensor,
                        v_diag.offset,
                        [v_diag.ap[0], [16, 16], [1, 16]],
                    )
                    nc.vector.tensor_tensor(vdiag3, mask3, vstat3, op=ALU.mult)
                    yield

                    # --- PH_D: H + HT --------------------------------------
                    H_bf = []
                    for g in range(2):
                        ph = phb_pool.tile([NUM_CAP, D_IN], f32, tag="phb", name="ph")[0:16, :]
                        for t in range(8):
                            ot = 8 * g + t
                            nc.tensor.matmul(
                                ph[:],
                                v_diag[:, 16 * ot : 16 * (ot + 1)],
                                WT_bf[ot][:],
                                start=(t == 0),
                                stop=(t == 7),
                            )
                        hb = spool.tile(
                            [16, D_IN], bf16, tag=f"hbf{g}", name=f"hbf{g}"
                        )
                        nc.scalar.copy(out=hb[:], in_=ph[:])
                        H_bf.append(hb)
                    HT = [
                        spool.tile([128, NUM_CAP], bf16, tag=f"ht{kc}", name=f"ht{kc}")
                        for kc in range(KC)
                    ]
                    for g in range(2):
                        for kc in range(KC):
                            pht = pt_pool.tile([128, 16], bf16, tag="pt", name="pht")
                            nc.tensor.transpose(
                                pht[:],
                                H_bf[g][:, 128 * kc : 128 * (kc + 1)],
                                I128[0:16, 0:16],
                            )
                            nc.scalar.copy(
                                out=HT[kc][:, 16 * g : 16 * (g + 1)], in_=pht[:]
                            )
                    yield

                    # --- PH_E: b, softmax, cT ------------------------------
                    pb = phb_pool.tile([NUM_CAP, S], f32, tag="phb", name="pb")
                    for kc in range(KC):
                        nc.tensor.matmul(
                            pb[:],
                            HT[kc][:],
                            uT[kc][:],
                            start=(kc == 0),
                            stop=(kc == KC - 1),
                        )
                    e_bf = spool.tile([NUM_CAP, S], bf16, tag="e_bf", name="e_bf")
                    if EXP_SCALE_AP:
                        nc.scalar.activation(
                            out=e_bf[:], in_=pb[:], func=AF.Exp, scale=rs[:]
                        )
                    else:
                        b_sc = spool.tile([NUM_CAP, S], f32, tag="b_sc", name="b_sc")
                        nc.vector.tensor_scalar_mul(b_sc[:], pb[:], rs[:])
                        nc.scalar.activation(out=e_bf[:], in_=b_sc[:], func=AF.Exp)
                    cT = []
                    for ic in range(IC):
                        pet = pt_pool.tile(
                            [128, NUM_CAP], bf16, tag="pt", name="pet"
                        )
                        nc.tensor.transpose(
                            pet[:],
                            e_bf[:, 128 * ic : 128 * (ic + 1)],
                            I128[0:NUM_CAP, 0:NUM_CAP],
                        )
                        et = spool.tile(
                            [128, NUM_CAP], bf16, tag=f"et{ic}", name=f"et{ic}"
                        )
                        nc.scalar.copy(out=et[:], in_=pet[:])
                        z = spool.tile([128, 1], f32, tag=f"z{ic}", name=f"z{ic}")
                        nc.vector.reduce_sum(z[:], et[:], axis=mybir.AxisListType.X)
                        rz = spool.tile([128, 1], f32, tag=f"rz{ic}", name=f"rz{ic}")
                        nc.vector.reciprocal(rz[:], z[:])
                        ct = spool.tile(
                            [128, NUM_CAP], bf16, tag=f"ct{ic}", name=f"ct{ic}"
                        )
                        nc.vector.tensor_scalar_mul(ct[:], et[:], rz[:])
                        cT.append(ct)
                    yield

            # ---- software-pipelined pair loop ----------------------------
            # process batches in pairs; the two routing chains interleave so
            # one chain's PE matmuls fill the other's DMA/DVE stalls, and the
            # next pair's projection fills the rest.
            import itertools

            NPAIR = BPC // 2
            cur = [emit_loads(0), emit_loads(1)]
            for (uTx, uhx) in cur:
                for c in proj_chunks(uTx, uhx):
                    c()
            for t in range(NPAIR):
                (uT_a, uh_a), (uT_b, uh_b) = cur
                if t + 1 < NPAIR:
                    nxt = [emit_loads(2 * t + 2), emit_loads(2 * t + 3)]
                    chunks = []
                    for (uTx, uhx) in nxt:
                        chunks.extend(proj_chunks(uTx, uhx))
                else:
                    nxt = None
                    chunks = []
                gens = [
                    routing(2 * t, uT_a, uh_a),
                    routing(2 * t + 1, uT_b, uh_b),
                ]
                n_yields = 22
                i = 0
                done = 0
                for _ in itertools.chain.from_iterable(
                    itertools.zip_longest(*gens)
                ):
                    i += 1
                    want = min(len(chunks) + done, (i * 8) // n_yields)
                    while done < want and chunks:
                        chunks.pop(0)()
                        done += 1
                for c in chunks:
                    c()
                cur = nxt

    nc.compile()
    return nc


def _get_program():
    if "nc" not in _PROGRAM_CACHE:
        _PROGRAM_CACHE["nc"] = _build_program()
    return _PROGRAM_CACHE["nc"]


def _prep_inputs(u_vecs, W):
    uT_all = np.ascontiguousarray(u_vecs.transpose(0, 2, 1).astype(np.float32))
    W2 = np.ascontiguousarray(W.reshape(D_IN, D_OUT).astype(np.float32))
    WT = np.ascontiguousarray(W2.T)
    consts = _build_consts_np()
    in_maps = []
    for c in range(N_CORES):
        in_maps.append(
            {
                "uT": uT_all[c * BPC : (c + 1) * BPC],
                "W": W2,
                "WT": WT,
                "consts": consts,
            }
        )
    return in_maps


def kernel(u_vecs: np.ndarray, W: np.ndarray) -> np.ndarray:
    from concourse.bass_utils import run_bass_kernel_spmd

    nc = _get_program()
    in_maps = _prep_inputs(u_vecs, W)
    res = run_bass_kernel_spmd(nc, in_maps, list(range(N_CORES)))
    out = np.concatenate(
        [np.asarray(res.results[c]["out"]) for c in range(N_CORES)], axis=0
    )
    return out.astype(np.float32)



# revision 14
# speedup vs baseline: 1.2552x; 1.2552x over previous
"""Trainium2 Bass kernel for the Capsule routing module.

Computation (per batch b):
    u_hat = u_vecs[b] @ W[0]            # (512, 2048), o = n*64 + d
    3 rounds of dynamic routing over 32 capsules of dim 64:
        c = softmax_n(b_logits); v = squash(sum_i c * u_hat); b_logits = v . u_hat
Returns outputs (64, 32, 64).

Sharding: data-parallel over batch across 8 NeuronCores (8 batches/core),
W replicated.  All matmuls in bf16 with fp32 PSUM accumulation.

This version fuses routing across QUADS of 4 batches so the per-capsule
partition dim fills all 128 PE rows:
  - b-logit path: H = S-masked @ W^T with M=128 (4 batches x 32 caps) shared
    W^T stream, then b computed TRANSPOSED (bT[i, n]) so softmax over n is a
    free-axis reduce (no e-transposes, no per-chunk scalar copies).
  - 1/||S|| folds into the H psum->sbuf copy (per-partition activation scale),
    so exp is a single plain Exp over [128, 512].
  - squash stats in one tensor_tensor_reduce + one Rsqrt activation.
"""

import numpy as np

NUM_CAP = 32
DIM_CAP = 64
ROUTINGS = 3
EPS = 1e-7
B, S, D_IN = 64, 512, 512
D_OUT = NUM_CAP * DIM_CAP  # 2048
N_CORES = 8
BPC = B // N_CORES  # batches per core
G = 4               # batches fused per routing group (quad)

_PROGRAM_CACHE = {}

KC = D_IN // 128   # 4 k-chunks
IC = S // 128      # 4 i-chunks
OC = D_OUT // 512  # 4 o-chunks of 512
OT = D_OUT // 128  # 16 o-partition-tiles

MASK_COLS = OT * 32 * G  # 2048


def _build_consts_np():
    """[128, 128+32+2048] bf16 constants: I128 | cT0 (1/32) | MASK4."""
    import ml_dtypes

    c = np.zeros((128, 160 + MASK_COLS + 512), np.float32)
    c[:, :128] = np.eye(128, dtype=np.float32)
    c[:, 128:160] = 1.0 / NUM_CAP
    # MASK4[p, 128*ot + 32*bb + nn] = 1 iff nn//2 == ot and p//64 == nn%2
    for ot in range(OT):
        for bb in range(G):
            for j in (0, 1):
                nn = 2 * ot + j
                if nn >= NUM_CAP:
                    continue
                col = 160 + 128 * ot + 32 * bb + nn
                c[64 * j : 64 * (j + 1), col] = 1.0
    return c.astype(ml_dtypes.bfloat16)


def _build_program():
    import concourse.bass as bass
    import concourse.mybir as mybir
    from concourse import bacc
    from concourse.tile import TileContext

    f32 = mybir.dt.float32
    bf16 = mybir.dt.bfloat16
    AF = mybir.ActivationFunctionType
    ALU = mybir.AluOpType

    nc = bacc.Bacc("TRN2", debug=False)

    uT_d = nc.dram_tensor("uT", [BPC, D_IN, S], f32, kind="ExternalInput")
    W_d = nc.dram_tensor("W", [D_IN, D_OUT], f32, kind="ExternalInput")
    WT_d = nc.dram_tensor("WT", [D_OUT, D_IN], f32, kind="ExternalInput")
    consts_d = nc.dram_tensor(
        "consts", [128, 160 + MASK_COLS + 512], bf16, kind="ExternalInput"
    )
    out_d = nc.dram_tensor("out", [BPC, NUM_CAP, DIM_CAP], f32, kind="ExternalOutput")

    with TileContext(nc) as tc:
        with (
            tc.tile_pool(name="persist", bufs=1) as ppool,
            tc.tile_pool(name="ut", bufs=6) as utpool,
            tc.tile_pool(name="uh", bufs=6) as uhpool,
            tc.tile_pool(name="small", bufs=2) as spool,
            tc.tile_pool(name="pp", bufs=4, space="PSUM") as pp_pool,
            tc.tile_pool(name="psS", bufs=1, space="PSUM") as psS_pool,
            tc.tile_pool(name="hbt", bufs=2, space="PSUM") as hbt_pool,
        ):
            # ---- once-per-core loads -------------------------------------
            consts = ppool.tile([128, 160 + MASK_COLS + 512], bf16, tag="consts",
                                name="consts")
            nc.sync.dma_start(out=consts[:], in_=consts_d[:])
            I128 = consts[:, 0:128]
            CT0 = consts[:, 128:160]
            MASK4 = consts[:, 160 : 160 + MASK_COLS]
            ZERO512 = consts[:, 160 + MASK_COLS : 160 + MASK_COLS + 512]

            # two persistent psS banks (by batch parity): PH_A only ever
            # writes rows 32q; zero once so the full-tile copy reads zeros
            # elsewhere
            psS_banks = []
            for j in range(2):
                z = psS_pool.tile([128, 512], f32, tag=f"psS{j}", name=f"psS{j}")
                nc.vector.tensor_copy(z[:], ZERO512)
                psS_banks.append(z)

            W_bf = []
            for kc in range(KC):
                t = ppool.tile([128, D_OUT], bf16, tag=f"wbf{kc}", name=f"wbf{kc}")
                nc.gpsimd.dma_start(out=t[:], in_=W_d[128 * kc : 128 * (kc + 1), :])
                W_bf.append(t)

            WT_bf = []
            for ot in range(OT):
                t = ppool.tile([128, D_IN], bf16, tag=f"wt{ot}", name=f"wt{ot}")
                nc.gpsimd.dma_start(out=t[:], in_=WT_d[128 * ot : 128 * (ot + 1), :])
                WT_bf.append(t)

            def emit_loads(b):
                uT = []
                for kc in range(KC):
                    t = utpool.tile([128, S], bf16, tag=f"ut{kc}", name=f"ut{kc}")
                    nc.gpsimd.dma_start(
                        out=t[:], in_=uT_d[b, 128 * kc : 128 * (kc + 1), :]
                    )
                    uT.append(t)
                uh = [
                    uhpool.tile([128, D_OUT], bf16, tag=f"uh{ic}", name=f"uh{ic}")
                    for ic in range(IC)
                ]
                return uT, uh

            def proj_chunks(uT, u_hat):
                """4 closures (one per ic): kc-outer loop reuses each
                stationary for 4 consecutive matmuls into 4 psum banks."""

                def mk(ic):
                    def go():
                        pps = [
                            pp_pool.tile([128, 512], f32, tag="pp", name="pp")
                            for _ in range(OC)
                        ]
                        for kc in range(KC):
                            for oc in range(OC):
                                nc.tensor.matmul(
                                    pps[oc][:],
                                    uT[kc][:, 128 * ic : 128 * (ic + 1)],
                                    W_bf[kc][:, 512 * oc : 512 * (oc + 1)],
                                    start=(kc == 0),
                                    stop=(kc == KC - 1),
                                )
                        for oc in range(OC):
                            eng = nc.vector if oc % 2 == 0 else nc.scalar
                            if oc % 2 == 0:
                                eng.tensor_copy(
                                    u_hat[ic][:, 512 * oc : 512 * (oc + 1)],
                                    pps[oc][:],
                                )
                            else:
                                eng.copy(
                                    out=u_hat[ic][:, 512 * oc : 512 * (oc + 1)],
                                    in_=pps[oc][:],
                                )

                    return go

                return [mk(ic) for ic in range(IC)]

            def routing_quad(t, datas):
                """Generator: routing for batches 4t..4t+3, fused.
                Yields at pipeline stall points; yields "late" once the
                u_hat/uT buffers of the first two batches are free."""
                ct4 = None
                for it in range(ROUTINGS):
                    # --- PH_A: per-capsule weighted sums, per batch --------
                    S4 = spool.tile([128, DIM_CAP], f32, tag="S4", name="S4")
                    for bb in range(G):
                        uTb, uhb = datas[bb]
                        psS = psS_banks[bb % 2]
                        for r in range(8):
                            for ic in range(IC):
                                if it == 0:
                                    lhsT_all = CT0
                                    coff = 0
                                else:
                                    lhsT_all = ct4
                                    coff = 128 * ic + 32 * bb
                                for q in range(4):
                                    n = 8 * q + r
                                    nc.tensor.matmul(
                                        psS[32 * q : 32 * q + 1,
                                            64 * r : 64 * (r + 1)],
                                        lhsT_all[:, coff + n : coff + n + 1],
                                        uhb[ic][:, 64 * n : 64 * (n + 1)],
                                        start=(ic == 0),
                                        stop=(ic == IC - 1),
                                        tile_position=(0, 32 * q),
                                    )
                        s_sc = spool.tile([128, 512], f32, tag="s_sc", name="s_sc")
                        nc.scalar.copy(out=s_sc[:], in_=psS[:])
                        sc_src = bass.AP(
                            s_sc.tensor, s_sc.offset,
                            [[32 * s_sc.ap[0][0], 4], [DIM_CAP, 8], [1, DIM_CAP]],
                        )
                        nc.sync.dma_start(
                            out=S4[32 * bb : 32 * (bb + 1), :], in_=sc_src
                        )
                        if it == ROUTINGS - 1 and bb == 1:
                            yield "late"
                    yield

                    # --- PH_B: squash stats --------------------------------
                    sq4 = spool.tile([128, DIM_CAP], f32, tag="sq4", name="sq4")
                    ss4 = spool.tile([128, 1], f32, tag="ss4", name="ss4")
                    nc.vector.tensor_tensor(sq4[:], S4[:], S4[:], op=ALU.mult)
                    nc.vector.reduce_sum(ss4[:], sq4[:], axis=mybir.AxisListType.X)
                    ss_eps = spool.tile([128, 1], f32, tag="ss_eps", name="ss_eps")
                    nc.vector.tensor_scalar_add(ss_eps[:], ss4[:], EPS)
                    rinv4 = spool.tile([128, 1], f32, tag="rinv4", name="rinv4")
                    nc.vector.reciprocal(rinv4[:], ss_eps[:])
                    rs4 = spool.tile([128, 1], f32, tag="rs4", name="rs4")
                    nc.scalar.activation(out=rs4[:], in_=rinv4[:], func=AF.Sqrt)

                    if it == ROUTINGS - 1:
                        v4f = spool.tile([128, DIM_CAP], f32, tag="v4f", name="v4f")
                        nc.vector.tensor_scalar_mul(v4f[:], S4[:], rs4[:])
                        for bb in range(G):
                            nc.sync.dma_start(
                                out=out_d[G * t + bb],
                                in_=v4f[32 * bb : 32 * (bb + 1), :],
                            )
                        return

                    S4bf = spool.tile([128, DIM_CAP], bf16, tag="S4bf", name="S4bf")
                    nc.vector.tensor_copy(S4bf[:], S4[:])
                    yield

                    # --- PH_C: vstat / vdiag -------------------------------
                    pvt4 = hbt_pool.tile([64, 128], bf16, tag="hbt", name="pvt4")
                    nc.tensor.transpose(pvt4[:], S4bf[:], I128[:, :])
                    vT4s = spool.tile([64, 128], bf16, tag="vT4s", name="vT4s")
                    nc.scalar.copy(out=vT4s[:], in_=pvt4[:])
                    vstat4 = spool.tile([128, 128], bf16, tag="vstat4",
                                        name="vstat4")
                    nc.vector.tensor_copy(vstat4[0:64, :], pvt4[:])
                    nc.sync.dma_start(out=vstat4[64:128, :], in_=vT4s[:])
                    vdiag4 = spool.tile([128, MASK_COLS], bf16, tag="vdiag4",
                                        name="vdiag4")
                    # iterate (p, mm, ot) so the broadcast dim is last
                    mask3 = bass.AP(
                        MASK4.tensor, MASK4.offset,
                        [MASK4.ap[0], [1, 128], [128, OT]],
                    )
                    vstat3 = bass.AP(
                        vstat4.tensor, vstat4.offset,
                        [vstat4.ap[0], [1, 128], [0, OT]],
                    )
                    vdiag3 = bass.AP(
                        vdiag4.tensor, vdiag4.offset,
                        [vdiag4.ap[0], [1, 128], [128, OT]],
                    )
                    nc.vector.tensor_tensor(vdiag3, mask3, vstat3, op=ALU.mult)
                    yield

                    # --- PH_D: H (all 4 batches share the W^T stream) ------
                    H4ps = hbt_pool.tile([128, D_IN], f32, tag="hbt", name="H4ps")
                    for ot in range(OT):
                        nc.tensor.matmul(
                            H4ps[:],
                            vdiag4[:, 128 * ot : 128 * (ot + 1)],
                            WT_bf[ot][:],
                            start=(ot == 0),
                            stop=(ot == OT - 1),
                        )
                    # psum->sbuf copy with 1/||S|| folded in (per-partition)
                    hb4 = spool.tile([128, D_IN], bf16, tag="hb4", name="hb4")
                    nc.scalar.activation(
                        out=hb4[:], in_=H4ps[:], func=AF.Copy, scale=rs4[:]
                    )
                    yield

                    # --- HT transposes -------------------------------------
                    pht = hbt_pool.tile([128, D_IN], bf16, tag="hbt", name="pht")
                    for kc in range(KC):
                        nc.tensor.transpose(
                            pht[:, 128 * kc : 128 * (kc + 1)],
                            hb4[:, 128 * kc : 128 * (kc + 1)],
                            I128[:, :],
                        )
                    HT4 = spool.tile([128, D_IN], bf16, tag="HT4", name="HT4")
                    nc.scalar.copy(out=HT4[:], in_=pht[:])

                    # --- PH_E: bT (transposed logits) ----------------------
                    bT4 = hbt_pool.tile([128, 512], f32, tag="hbt", name="bT4")
                    for bb in range(G):
                        uTb, _ = datas[bb]
                        for ib in range(IC):
                            for kc in range(KC):
                                nc.tensor.matmul(
                                    bT4[:, 128 * ib + 32 * bb :
                                        128 * ib + 32 * bb + 32],
                                    uTb[kc][:, 128 * ib : 128 * (ib + 1)],
                                    HT4[:, 128 * kc + 32 * bb :
                                        128 * kc + 32 * (bb + 1)],
                                    start=(kc == 0),
                                    stop=(kc == KC - 1),
                                )
                    yield

                    # --- softmax over n (free axis), i-major ---------------
                    e4 = spool.tile([128, 512], bf16, tag="e4", name="e4")
                    nc.scalar.activation(out=e4[:], in_=bT4[:], func=AF.Exp)
                    e4_3d = bass.AP(
                        e4.tensor, e4.offset,
                        [e4.ap[0], [32, 16], [1, 32]],
                    )
                    z4 = spool.tile([128, 16], f32, tag="z4", name="z4")
                    nc.vector.reduce_sum(z4[:], e4_3d, axis=mybir.AxisListType.X)
                    rz4 = spool.tile([128, 16], f32, tag="rz4", name="rz4")
                    nc.vector.reciprocal(rz4[:], z4[:])
                    ct4 = spool.tile([128, 512], bf16, tag="ct4", name="ct4")
                    ct4_3d = bass.AP(
                        ct4.tensor, ct4.offset,
                        [ct4.ap[0], [32, 16], [1, 32]],
                    )
                    rz4_b = bass.AP(
                        rz4.tensor, rz4.offset,
                        [rz4.ap[0], [1, 16], [0, 32]],
                    )
                    nc.vector.tensor_tensor(ct4_3d, e4_3d, rz4_b, op=ALU.mult)
                    yield

            # ---- software-pipelined quad loop ----------------------------
            NQUAD = BPC // G
            datas = [emit_loads(b) for b in range(G)]
            chunks0 = []
            for (uTx, uhx) in datas:
                chunks0.extend(proj_chunks(uTx, uhx))
            for c in chunks0:
                c()
            for t in range(NQUAD):
                if t + 1 < NQUAD:
                    nxt = [emit_loads(G * (t + 1) + j) for j in range(G)]
                    chunks = []
                    for (uTx, uhx) in nxt:
                        chunks.extend(proj_chunks(uTx, uhx))
                else:
                    nxt = None
                    chunks = []
                n_yields = 14
                i = 0
                done = 0
                late = False
                for y in routing_quad(t, datas):
                    i += 1
                    if y == "late":
                        late = True
                    lim = len(chunks) + done if late else min(8, len(chunks) + done)
                    want = min(lim, (i * 16) // n_yields)
                    while done < want and chunks:
                        chunks.pop(0)()
                        done += 1
                for c in chunks:
                    c()
                datas = nxt

    nc.compile()
    return nc


def _get_program():
    if "nc" not in _PROGRAM_CACHE:
        _PROGRAM_CACHE["nc"] = _build_program()
    return _PROGRAM_CACHE["nc"]


def _prep_inputs(u_vecs, W):
    uT_all = np.ascontiguousarray(u_vecs.transpose(0, 2, 1).astype(np.float32))
    W2 = np.ascontiguousarray(W.reshape(D_IN, D_OUT).astype(np.float32))
    WT = np.ascontiguousarray(W2.T)
    consts = _build_consts_np()
    in_maps = []
    for c in range(N_CORES):
        in_maps.append(
            {
                "uT": uT_all[c * BPC : (c + 1) * BPC],
                "W": W2,
                "WT": WT,
                "consts": consts,
            }
        )
    return in_maps


def kernel(u_vecs: np.ndarray, W: np.ndarray) -> np.ndarray:
    from concourse.bass_utils import run_bass_kernel_spmd

    nc = _get_program()
    in_maps = _prep_inputs(u_vecs, W)
    res = run_bass_kernel_spmd(nc, in_maps, list(range(N_CORES)))
    out = np.concatenate(
        [np.asarray(res.results[c]["out"]) for c in range(N_CORES)], axis=0
    )
    return out.astype(np.float32)
